# revision 4
# baseline (speedup 1.0000x reference)
"""Trainium2 Bass kernel for nn_CrossPatchContextModule.

Math (per batch b):
    hi = x @ W1[:D];  hj = x @ W1[D:]
    scores[i,j] = W2 . relu(hi[i] + hj[j] + b1) + b2     (diag forced to 0)
    w = softmax(scores, axis=j)
    out = x + LN(w @ x @ Wp + bp) * gamma + beta

Sharding: data-parallel over batch. B=8 batches -> 8 NeuronCores, one
batch per core, all parameters replicated. No collectives.

Per-core algorithm (N=D=256, P=128 partitions):
  * Fold a = |W2| into W1 on the host (W1' = W1 * a), so the pairwise relu
    tile R[e, (i,j)] = relu(a_e*(hi+hj+b1)) = a_e*relu(hi+hj+b1) is produced
    in ONE tensor_scalar (DVE) / activation-Relu (ACT) op per (i, e-chunk):
    in0 = hjbW (j on free axis), per-partition scalar = hiW[:, i].
  * scores[i, :] = sum_e sign(W2)_e * R[e, :]. Contract on the PE with a
    shifted-window one-hot weight: sigB is [128, 256] with column 128 equal
    to sign(W2) for that e-chunk and zeros elsewhere; sigB[:, 128-p:256-p]
    is a [128,128] matrix whose only nonzero column is p, so the matvec
    result lands on PSUM partition p while accumulating +0 onto all other
    rows. All 256 matmuls form one accumulation group into a single PSUM
    bank => scores end up dense [i(part), j(free)] with rows (p, p+128)
    packed as the two 256-wide halves of the free axis.
  * softmax: evacuate + add b2 (DVE tensor_scalar from PSUM), multiply a
    hostside diag-zero mask, ACT Exp with accum_out giving row sums for
    free, reciprocal + per-partition scale.
  * ctx^T = x^T(chunks as lhsT) @ w^T (w transposed 128x128 via PE),
    proj = ctx^T(lhsT) @ Wp -> [i(part), e(free)], + bp, LayerNorm via
    bn_stats/bn_aggr, rstd = Exp(-0.5*Ln(var+eps)), residual add.

R tiles are fp16 (PE runs fp32 matmuls at 1/4 speed; fp16 streams at
1 col/cycle and DVE tensor_scalar gets 4x mode). Everything else fp32.
"""

import numpy as np
from contextlib import ExitStack

import concourse.bass as bass
import concourse.bacc as bacc
import concourse.tile as tile
from concourse import mybir
from concourse.bass_utils import run_bass_kernel_spmd

B, N, D = 8, 256, 256
P = 128
LN_EPS = 1e-5
F32 = mybir.dt.float32
F16 = mybir.dt.float16
AF = mybir.ActivationFunctionType
OP = mybir.AluOpType

# rows whose pairwise tiles are produced on ACT instead of DVE (balance:
# DVE ~127ns/op vs ACT ~400ns/op; DVE takes 3 of every 4 rows)
def _row_on_act(p):
    return (p % 4) == 3


def _build_program(b2_val: float, use_gamma: bool):
    nc = bacc.Bacc("TRN2", target_bir_lowering=False, debug=False)

    xb_d = nc.dram_tensor("xb", [N, D], F32, kind="ExternalInput")
    xpb_d = nc.dram_tensor("xpb", [N, D], F32, kind="ExternalInput")
    w1a_d = nc.dram_tensor("w1a", [D, D], F32, kind="ExternalInput")
    w1b_d = nc.dram_tensor("w1b", [D, D], F32, kind="ExternalInput")
    ab1_d = nc.dram_tensor("ab1c", [P, 2], F32, kind="ExternalInput")
    sb0_d = nc.dram_tensor("sb0", [P, 2 * P], F16, kind="ExternalInput")
    sb1_d = nc.dram_tensor("sb1", [P, 2 * P], F16, kind="ExternalInput")
    mask_d = nc.dram_tensor("maskd", [P, N * 2], F32, kind="ExternalInput")
    ident_d = nc.dram_tensor("ident", [P, P], F32, kind="ExternalInput")
    wp_d = nc.dram_tensor("wp", [D, D], F32, kind="ExternalInput")
    bpr_d = nc.dram_tensor("bpr", [P, D], F32, kind="ExternalInput")
    gam_d = (
        nc.dram_tensor("gamr", [P, D], F32, kind="ExternalInput")
        if use_gamma
        else None
    )
    out_d = nc.dram_tensor("out", [N, D], F32, kind="ExternalOutput")

    with tile.TileContext(nc) as tc, ExitStack() as ctx:
        const = ctx.enter_context(tc.tile_pool(name="const", bufs=1))
        rpool = ctx.enter_context(tc.tile_pool(name="rtiles", bufs=3))
        ppre = ctx.enter_context(
            tc.tile_pool(name="ppre", bufs=2, space="PSUM")
        )
        pscore = ctx.enter_context(
            tc.tile_pool(name="pscore", bufs=1, space="PSUM")
        )

        # per-partition scalar constants for activation bias operands
        zero1 = const.tile([P, 1], F32)
        nc.vector.memset(zero1, 0.0)
        eps1 = const.tile([P, 1], F32)
        nc.vector.memset(eps1, LN_EPS)

        # ---------------- input DMAs needed before the main loop ----------
        ident = const.tile([P, P], F32)
        nc.sync.dma_start(ident, ident_d[:])
        x = [const.tile([P, D], F32, tag=f"x{c}", name=f"x{c}") for c in range(2)]
        for c in range(2):
            nc.sync.dma_start(x[c], xb_d[c * P : (c + 1) * P, :])
        w1a = [const.tile([P, D], F32, tag=f"w1a{c}", name=f"w1a{c}") for c in range(2)]
        w1b = [const.tile([P, D], F32, tag=f"w1b{c}", name=f"w1b{c}") for c in range(2)]
        for c in range(2):
            nc.sync.dma_start(w1a[c], w1a_d[c * P : (c + 1) * P, :])
            nc.sync.dma_start(w1b[c], w1b_d[c * P : (c + 1) * P, :])
        ab1c = const.tile([P, 2], F32)
        nc.sync.dma_start(ab1c, ab1_d[:])
        sb = [const.tile([P, 2 * P], F16, tag=f"sb{c}", name=f"sb{c}") for c in range(2)]
        nc.sync.dma_start(sb[0], sb0_d[:])
        nc.sync.dma_start(sb[1], sb1_d[:])

        # ---------------- x^T via PE transpose ----------------------------
        xT = [const.tile([P, N], F32, tag=f"xT{c}", name=f"xT{c}") for c in range(2)]
        for ic in range(2):
            for dc in range(2):
                tp = ppre.tile([P, P], F32, tag="tp")
                nc.tensor.transpose(tp, x[ic][:, dc * P : (dc + 1) * P], ident)
                nc.scalar.copy(xT[dc][:, ic * P : (ic + 1) * P], tp)

        # ---------------- hiW (fp32 scalars), hjbW (fp16 stream) ----------
        # hiW[e,i] = sum_d (W1a*a)[d,e] x[i,d] ; hjbW[e,j] = ... + a*b1
        hiW = [const.tile([P, N], F32, tag=f"hiW{c}", name=f"hiW{c}") for c in range(2)]
        hjbW = [const.tile([P, N], F16, tag=f"hjbW{c}", name=f"hjbW{c}") for c in range(2)]
        for ec in range(2):
            ph = ppre.tile([P, N], F32, tag="mm")
            for dc in range(2):
                nc.tensor.matmul(
                    ph,
                    w1a[dc][:, ec * P : (ec + 1) * P],
                    xT[dc],
                    start=(dc == 0),
                    stop=(dc == 1),
                )
            nc.vector.tensor_copy(hiW[ec], ph)
        for ec in range(2):
            ph = ppre.tile([P, N], F32, tag="mm")
            for dc in range(2):
                nc.tensor.matmul(
                    ph,
                    w1b[dc][:, ec * P : (ec + 1) * P],
                    xT[dc],
                    start=(dc == 0),
                    stop=(dc == 1),
                )
            # + a*b1 (per-partition bias) while converting to fp16
            nc.scalar.activation(
                hjbW[ec], ph, AF.Identity, bias=ab1c[:, ec : ec + 1]
            )

        # ---------------- pairwise scores --------------------------------
        # psum_s[p, h*256+j] = scores[i=p+128h, j]
        psum_s = pscore.tile([P, 2 * N], F32)
        for p in range(P):
            R = [rpool.tile([P, 2 * N], F16, tag=f"R{c}", name=f"R{c}") for c in range(2)]
            on_act = _row_on_act(p)
            for c in range(2):
                for h in range(2):
                    i = p + P * h
                    dst = R[c][:, h * N : (h + 1) * N]
                    if on_act:
                        nc.scalar.activation(
                            dst, hjbW[c], AF.Relu, bias=hiW[c][:, i : i + 1]
                        )
                    else:
                        nc.vector.tensor_scalar(
                            out=dst,
                            in0=hjbW[c],
                            scalar1=hiW[c][:, i : i + 1],
                            scalar2=0.0,
                            op0=OP.add,
                            op1=OP.max,
                        )
            nc.tensor.matmul(
                psum_s,
                sb[0][:, P - p : 2 * P - p],
                R[0],
                start=(p == 0),
                stop=False,
            )
            nc.tensor.matmul(
                psum_s,
                sb[1][:, P - p : 2 * P - p],
                R[1],
                start=False,
                stop=(p == P - 1),
            )

        # ---------------- epilogue-only input DMAs ------------------------
        maskd = const.tile([P, 2 * N], F32)
        nc.sync.dma_start(maskd, mask_d[:])
        wp = [const.tile([P, D], F32, tag=f"wp{c}", name=f"wp{c}") for c in range(2)]
        for c in range(2):
            nc.sync.dma_start(wp[c], wp_d[c * P : (c + 1) * P, :])
        bpr = const.tile([P, D], F32)
        nc.sync.dma_start(bpr, bpr_d[:])
        xpb = [const.tile([P, D], F32, tag=f"xpb{c}", name=f"xpb{c}") for c in range(2)]
        for c in range(2):
            nc.sync.dma_start(xpb[c], xpb_d[c * P : (c + 1) * P, :])
        if use_gamma:
            gam = const.tile([P, D], F32)
            nc.sync.dma_start(gam, gam_d[:])

        # ---------------- softmax ----------------------------------------
        sm = const.tile([P, 2 * N], F32)
        nc.vector.tensor_scalar(
            out=sm, in0=psum_s, scalar1=b2_val, scalar2=None, op0=OP.add
        )
        sm2 = const.tile([P, 2 * N], F32)
        nc.vector.tensor_tensor(out=sm2, in0=sm, in1=maskd, op=OP.mult)
        ew = const.tile([P, 2 * N], F32)
        S = const.tile([P, 2], F32)
        for h in range(2):
            nc.scalar.activation(
                ew[:, h * N : (h + 1) * N],
                sm2[:, h * N : (h + 1) * N],
                AF.Exp,
                bias=zero1[:, 0:1],
                accum_out=S[:, h : h + 1],
            )
        recip = const.tile([P, 2], F32)
        nc.vector.reciprocal(recip, S)
        wgt = const.tile([P, 2 * N], F32)
        for h in range(2):
            nc.vector.tensor_scalar(
                out=wgt[:, h * N : (h + 1) * N],
                in0=ew[:, h * N : (h + 1) * N],
                scalar1=recip[:, h : h + 1],
                scalar2=None,
                op0=OP.mult,
            )

        # ---------------- w^T via PE transpose ---------------------------
        wT = [const.tile([P, N], F32, tag=f"wT{c}", name=f"wT{c}") for c in range(2)]
        for ci in range(2):
            for cj in range(2):
                tp = ppre.tile([P, P], F32, tag="tp")
                nc.tensor.transpose(
                    tp, wgt[:, ci * N + cj * P : ci * N + (cj + 1) * P], ident
                )
                nc.scalar.copy(wT[cj][:, ci * P : (ci + 1) * P], tp)

        # ---------------- ctx^T[d,i] = sum_j x[j,d] w[i,j] ----------------
        ctxT = [const.tile([P, N], F32, tag=f"ctxT{c}", name=f"ctxT{c}") for c in range(2)]
        for dc in range(2):
            pc = ppre.tile([P, N], F32, tag="mm")
            for jc in range(2):
                nc.tensor.matmul(
                    pc,
                    x[jc][:, dc * P : (dc + 1) * P],
                    wT[jc],
                    start=(jc == 0),
                    stop=(jc == 1),
                )
            nc.scalar.copy(ctxT[dc], pc)

        # ---------------- proj / LayerNorm / residual ---------------------
        for icc in range(2):
            pp = ppre.tile([P, N], F32, tag="mm")
            for dc in range(2):
                nc.tensor.matmul(
                    pp,
                    ctxT[dc][:, icc * P : (icc + 1) * P],
                    wp[dc],
                    start=(dc == 0),
                    stop=(dc == 1),
                )
            pb = const.tile([P, D], F32, tag=f"pb{icc}")
            nc.vector.tensor_tensor(out=pb, in0=pp, in1=bpr, op=OP.add)
            st = const.tile([P, 6], F32, tag=f"st{icc}")
            nc.vector.bn_stats(st, pb)
            mv = const.tile([P, 2], F32, tag=f"mv{icc}")
            nc.vector.bn_aggr(mv, st)
            lnv = const.tile([P, 1], F32, tag=f"lnv{icc}")
            nc.scalar.activation(lnv, mv[:, 1:2], AF.Ln, bias=eps1[:, 0:1])
            rstd = const.tile([P, 1], F32, tag=f"rstd{icc}")
            nc.scalar.activation(rstd, lnv, AF.Exp, bias=zero1[:, 0:1], scale=-0.5)
            nmr = const.tile([P, 1], F32, tag=f"nmr{icc}")
            nc.vector.tensor_scalar(
                out=nmr,
                in0=mv[:, 0:1],
                scalar1=rstd[:, 0:1],
                scalar2=-1.0,
                op0=OP.mult,
                op1=OP.mult,
            )
            tt = const.tile([P, D], F32, tag=f"tt{icc}")
            nc.vector.tensor_scalar(
                out=tt,
                in0=pb,
                scalar1=rstd[:, 0:1],
                scalar2=nmr[:, 0:1],
                op0=OP.mult,
                op1=OP.add,
            )
            if use_gamma:
                tg = const.tile([P, D], F32, tag=f"tg{icc}")
                nc.vector.tensor_tensor(out=tg, in0=tt, in1=gam, op=OP.mult)
                tt = tg
            ot = const.tile([P, D], F32, tag=f"ot{icc}")
            nc.vector.tensor_tensor(out=ot, in0=tt, in1=xpb[icc], op=OP.add)
            nc.sync.dma_start(out_d[icc * P : (icc + 1) * P, :], ot)

    nc.compile()
    return nc


_cache = {}


def _get_program(b2_val: float, use_gamma: bool):
    key = (b2_val, use_gamma)
    if key not in _cache:
        _cache[key] = _build_program(b2_val, use_gamma)
    return _cache[key]


def _host_inputs(inputs):
    x = np.ascontiguousarray(np.asarray(inputs["patch_features"], np.float32))
    W1 = np.asarray(inputs["W1"], np.float32)
    b1 = np.asarray(inputs["b1"], np.float32)
    W2 = np.asarray(inputs["W2"], np.float32).reshape(-1)
    b2 = float(np.asarray(inputs["b2"], np.float32).reshape(-1)[0])
    Wp = np.ascontiguousarray(np.asarray(inputs["Wp"], np.float32))
    bp = np.asarray(inputs["bp"], np.float32)
    gam = np.asarray(inputs["ln_gamma"], np.float32)
    bet = np.asarray(inputs["ln_beta"], np.float32)

    a = np.abs(W2)
    sig = np.where(W2 >= 0.0, 1.0, -1.0).astype(np.float32)
    w1a = np.ascontiguousarray(W1[:D] * a[None, :])
    w1b = np.ascontiguousarray(W1[D:] * a[None, :])
    ab1c = np.ascontiguousarray((a * b1).reshape(2, P).T)  # [P, 2]
    sbs = []
    for c in range(2):
        m = np.zeros((P, 2 * P), np.float16)
        m[:, P] = sig[c * P : (c + 1) * P].astype(np.float16)
        sbs.append(m)
    mask = np.ones((P, 2 * N), np.float32)
    for p in range(P):
        mask[p, p] = 0.0
        mask[p, N + P + p] = 0.0
    ident = np.eye(P, dtype=np.float32)
    bpr = np.ascontiguousarray(np.broadcast_to(bp[None, :], (P, D)))
    xpb = x + bet[None, None, :]
    use_gamma = not np.all(gam == 1.0)
    gamr = np.ascontiguousarray(np.broadcast_to(gam[None, :], (P, D)))

    common = {
        "w1a": w1a,
        "w1b": w1b,
        "ab1c": ab1c,
        "sb0": sbs[0],
        "sb1": sbs[1],
        "maskd": mask,
        "ident": ident,
        "wp": Wp,
        "bpr": bpr,
    }
    if use_gamma:
        common["gamr"] = gamr
    in_maps = []
    for b in range(B):
        m = dict(common)
        m["xb"] = np.ascontiguousarray(x[b])
        m["xpb"] = np.ascontiguousarray(xpb[b])
        in_maps.append(m)
    return in_maps, b2, use_gamma


def _run(inputs, trace=False, tmpdir=None):
    in_maps, b2, use_gamma = _host_inputs(inputs)
    nc = _get_program(b2, use_gamma)
    res = run_bass_kernel_spmd(
        nc, in_maps, list(range(B)), trace=trace, tmpdir=tmpdir
    )
    out = np.stack([res.results[b]["out"] for b in range(B)]).astype(np.float32)
    return out, res


def kernel(**inputs) -> np.ndarray:
    out, _ = _run(inputs)
    return out


def predicted_time_ns():
    """Cost-model timeline estimate of one core's NEFF execution (ns)."""
    from concourse.timeline_sim import TimelineSim

    assert _cache, "run the kernel first"
    nc = next(iter(_cache.values()))
    tl = TimelineSim(nc, trace=False)
    return int(tl.simulate())


def timeline_trace(path="timeline.pftrace"):
    """Dump a perfetto trace of the cost-model timeline."""
    from concourse.timeline_sim import TimelineSim

    assert _cache, "run the kernel first"
    nc = next(iter(_cache.values()))
    tl = TimelineSim(nc, trace=True)
    t = tl.simulate()
    tl.perfetto.save(path)
    return t


# revision 9
# speedup vs baseline: 1.2917x; 1.2917x over previous
"""Trainium2 Bass kernel for nn_CrossPatchContextModule.

Math (per batch b):
    hi = x @ W1[:D];  hj = x @ W1[D:]
    scores[i,j] = W2 . relu(hi[i] + hj[j] + b1) + b2     (diag forced to 0)
    w = softmax(scores, axis=j)
    out = x + LN(w @ x @ Wp + bp) * gamma + beta

Sharding: data-parallel over batch. B=8 batches -> 8 NeuronCores, one
batch per core, all parameters replicated. No collectives.

Per-core algorithm (N=D=256, P=128 partitions):
  * Fold a = |W2| into W1 on the host (W1' = W1 * a), so the pairwise relu
    tile R[e, (i,j)] = relu(a_e*(hi+hj+b1)) = a_e*relu(hi+hj+b1) is produced
    in ONE tensor_scalar (DVE) / activation-Relu (ACT) op per (i, e-chunk):
    in0 = hjbW (j on free axis), per-partition scalar = hiW[:, i].
  * scores[i, :] = sum_e sign(W2)_e * R[e, :]. Contract on the PE with a
    shifted-window one-hot weight: sb32 is [128, 64] with column 32 equal
    to sign(W2) for that e-chunk and zeros elsewhere; sb32[:, 32-m:64-m]
    is a [128,32] matrix whose only nonzero column is m = p%32, so with the
    output sliced to partitions [32g, 32g+32) (g = p//32, tile_position
    col-group g) the matvec lands on PSUM partition p while accumulating +0
    onto the other rows of its group. Rows are emitted round-robin over the
    4 column groups so consecutive matmuls execute concurrently in disjoint
    32-column strips of the PE array. All 256 matmuls form one accumulation
    group into a single PSUM bank => scores end up dense [i(part), j(free)]
    with rows (p, p+128) packed as the two 256-wide halves of the free axis.
  * softmax: ACT Identity(+b2) evacuation, diag-zero mask multiply (Pool),
    ACT Exp with accum_out giving row sums for free, reciprocal +
    per-partition scale (DVE).
  * ctx^T = x(chunks as lhsT) @ w^T (w transposed 128x128 via PE),
    proj = ctx^T(lhsT) @ Wp -> [i(part), e(free)], + bp, LayerNorm via
    bn_stats/bn_aggr, rstd = Exp(-0.5*Ln(var+eps)), residual add.

R tiles and the hi/hj matmul operands are fp16 (PE runs fp32 matmuls at
1/4 speed; fp16 streams at 1 col/cycle and DVE tensor_scalar gets the
16-bit perf mode). scores/softmax/ctx/proj stay fp32.
"""

import numpy as np
from contextlib import ExitStack

import concourse.bass as bass
import concourse.bacc as bacc
import concourse.tile as tile
from concourse import mybir
from concourse.bass_utils import run_bass_kernel_spmd

B, N, D = 8, 256, 256
P = 128
LN_EPS = 1e-5
F32 = mybir.dt.float32
F16 = mybir.dt.float16
AF = mybir.ActivationFunctionType
OP = mybir.AluOpType

# Rows p >= 96 (column-group 3) produce their relu tiles on ACT, the rest
# on DVE (HW: DVE ~127ns/op at 16-bit perf mode vs ACT ~400ns/op -> 96/32).
ACT_GROUP = 3


def _build_program(b2_val: float, use_gamma: bool, use_beta: bool):
    nc = bacc.Bacc("TRN2", target_bir_lowering=False, debug=False)

    xb_d = nc.dram_tensor("xb", [N, D], F32, kind="ExternalInput")
    w1a_d = nc.dram_tensor("w1a", [D, D], F16, kind="ExternalInput")
    w1b_d = nc.dram_tensor("w1b", [D, D], F16, kind="ExternalInput")
    ab1_d = nc.dram_tensor("ab1c", [P, 2], F32, kind="ExternalInput")
    sb0_d = nc.dram_tensor("sb0", [P, 64], F16, kind="ExternalInput")
    sb1_d = nc.dram_tensor("sb1", [P, 64], F16, kind="ExternalInput")
    mask_d = nc.dram_tensor("maskd", [P, N * 2], F32, kind="ExternalInput")
    ident_d = nc.dram_tensor("ident", [P, P], F32, kind="ExternalInput")
    wp_d = nc.dram_tensor("wp", [D, D], F32, kind="ExternalInput")
    bpr_d = nc.dram_tensor("bpr", [P, D], F32, kind="ExternalInput")
    xpb_d = (
        nc.dram_tensor("xpb", [N, D], F32, kind="ExternalInput")
        if use_beta
        else None
    )
    gam_d = (
        nc.dram_tensor("gamr", [P, D], F32, kind="ExternalInput")
        if use_gamma
        else None
    )
    out_d = nc.dram_tensor("out", [N, D], F32, kind="ExternalOutput")

    with tile.TileContext(nc) as tc, ExitStack() as ctx:
        const = ctx.enter_context(tc.tile_pool(name="const", bufs=1))
        rpool = ctx.enter_context(tc.tile_pool(name="rtiles", bufs=8))
        ppre = ctx.enter_context(tc.tile_pool(name="ppre", bufs=2, space="PSUM"))
        pscore = ctx.enter_context(
            tc.tile_pool(name="pscore", bufs=1, space="PSUM")
        )

        # per-partition scalar constants for activation bias operands
        zero1 = const.tile([P, 1], F32)
        nc.vector.memset(zero1, 0.0)
        eps1 = const.tile([P, 1], F32)
        nc.vector.memset(eps1, LN_EPS)
        b2v = const.tile([P, 1], F32)
        nc.vector.memset(b2v, b2_val)

        # ------- input DMAs needed before the main loop (sync queue) ------
        ident = const.tile([P, P], F32)
        nc.sync.dma_start(ident, ident_d[:])
        x = [const.tile([P, D], F32, tag=f"x{c}", name=f"x{c}") for c in range(2)]
        for c in range(2):
            nc.sync.dma_start(x[c], xb_d[c * P : (c + 1) * P, :])
        w1a = [const.tile([P, D], F16, tag=f"w1a{c}", name=f"w1a{c}") for c in range(2)]
        w1b = [const.tile([P, D], F16, tag=f"w1b{c}", name=f"w1b{c}") for c in range(2)]
        for c in range(2):
            nc.sync.dma_start(w1a[c], w1a_d[c * P : (c + 1) * P, :])
            nc.sync.dma_start(w1b[c], w1b_d[c * P : (c + 1) * P, :])
        ab1c = const.tile([P, 2], F32)
        nc.sync.dma_start(ab1c, ab1_d[:])
        sb = [const.tile([P, 64], F16, tag=f"sb{c}", name=f"sb{c}") for c in range(2)]
        nc.sync.dma_start(sb[0], sb0_d[:])
        nc.sync.dma_start(sb[1], sb1_d[:])

        # ---------------- x^T via PE transpose (fp16 out) -----------------
        xT = [const.tile([P, N], F16, tag=f"xT{c}", name=f"xT{c}") for c in range(2)]
        for ic in range(2):
            for dc in range(2):
                tp = ppre.tile([P, P], F32, tag="tp")
                nc.tensor.transpose(tp, x[ic][:, dc * P : (dc + 1) * P], ident)
                nc.scalar.copy(xT[dc][:, ic * P : (ic + 1) * P], tp)

        # ---------------- hiW (fp32 scalars), hjbW (fp16 stream) ----------
        # hiW[e,i] = sum_d (W1a*a)[d,e] x[i,d] ; hjbW[e,j] = ... + a*b1
        hiW = [const.tile([P, N], F32, tag=f"hiW{c}", name=f"hiW{c}") for c in range(2)]
        hjbW = [const.tile([P, N], F16, tag=f"hjbW{c}", name=f"hjbW{c}") for c in range(2)]
        for ec in range(2):
            ph = ppre.tile([P, N], F32, tag="mm")
            for dc in range(2):
                nc.tensor.matmul(
                    ph,
                    w1a[dc][:, ec * P : (ec + 1) * P],
                    xT[dc],
                    start=(dc == 0),
                    stop=(dc == 1),
                )
            nc.vector.tensor_copy(hiW[ec], ph)
        for ec in range(2):
            ph = ppre.tile([P, N], F32, tag="mm")
            for dc in range(2):
                nc.tensor.matmul(
                    ph,
                    w1b[dc][:, ec * P : (ec + 1) * P],
                    xT[dc],
                    start=(dc == 0),
                    stop=(dc == 1),
                )
            # + a*b1 (per-partition bias) while converting to fp16
            nc.scalar.activation(
                hjbW[ec], ph, AF.Identity, bias=ab1c[:, ec : ec + 1]
            )

        # ---------------- pairwise scores --------------------------------
        # psum_s[p, h*256+j] = scores[i=p+128h, j]
        # Rb layout: [c0h0 | c0h1 | c1h0 | c1h1], each 256 wide.
        psum_s = pscore.tile([P, 2 * N], F32)
        for q in range(32):
            rows = [q, q + 32, q + 64, q + 96]
            rbs = []
            for k, p in enumerate(rows):
                rb = rpool.tile([P, 4 * N], F16, tag="Rb", name=f"Rb{p}")
                on_act = (p >> 5) == ACT_GROUP
                for c in range(2):
                    for h in range(2):
                        i = p + P * h
                        dst = rb[:, (c * 2 + h) * N : (c * 2 + h + 1) * N]
                        if on_act:
                            nc.scalar.activation(
                                dst, hjbW[c], AF.Relu, bias=hiW[c][:, i : i + 1]
                            )
                        else:
                            nc.vector.tensor_scalar(
                                out=dst,
                                in0=hjbW[c],
                                scalar1=hiW[c][:, i : i + 1],
                                scalar2=0.0,
                                op0=OP.add,
                                op1=OP.max,
                            )
                rbs.append(rb)
            for c in range(2):
                for k, p in enumerate(rows):
                    m = p & 31
                    nc.tensor.matmul(
                        psum_s[32 * k : 32 * (k + 1), :],
                        sb[c][:, 32 - m : 64 - m],
                        rbs[k][:, c * 2 * N : (c * 2 + 2) * N],
                        start=(q == 0 and c == 0),
                        stop=(q == 31 and c == 1),
                        tile_position=(0, 32 * k),
                        skip_group_check=True,
                    )

        # ------- epilogue-only input DMAs (gpsimd queue, off-critical) ----
        maskd = const.tile([P, 2 * N], F32)
        nc.gpsimd.dma_start(maskd, mask_d[:])
        wp = [const.tile([P, D], F32, tag=f"wp{c}", name=f"wp{c}") for c in range(2)]
        for c in range(2):
            nc.gpsimd.dma_start(wp[c], wp_d[c * P : (c + 1) * P, :])
        bpr = const.tile([P, D], F32)
        nc.gpsimd.dma_start(bpr, bpr_d[:])
        if use_beta:
            xpb = [
                const.tile([P, D], F32, tag=f"xpb{c}", name=f"xpb{c}")
                for c in range(2)
            ]
            for c in range(2):
                nc.gpsimd.dma_start(xpb[c], xpb_d[c * P : (c + 1) * P, :])
        else:
            xpb = x
        if use_gamma:
            gam = const.tile([P, D], F32)
            nc.gpsimd.dma_start(gam, gam_d[:])

        # ---------------- softmax ----------------------------------------
        sm = const.tile([P, 2 * N], F32)
        nc.scalar.activation(sm, psum_s, AF.Identity, bias=b2v[:, 0:1])
        sm2 = const.tile([P, 2 * N], F32)
        nc.gpsimd.tensor_tensor(out=sm2, in0=sm, in1=maskd, op=OP.mult)
        ew = const.tile([P, 2 * N], F32)
        S = const.tile([P, 2], F32)
        for h in range(2):
            nc.scalar.activation(
                ew[:, h * N : (h + 1) * N],
                sm2[:, h * N : (h + 1) * N],
                AF.Exp,
                bias=zero1[:, 0:1],
                accum_out=S[:, h : h + 1],
            )
        recip = const.tile([P, 2], F32)
        nc.vector.reciprocal(recip, S)
        wgt = const.tile([P, 2 * N], F32)
        for h in range(2):
            nc.vector.tensor_scalar(
                out=wgt[:, h * N : (h + 1) * N],
                in0=ew[:, h * N : (h + 1) * N],
                scalar1=recip[:, h : h + 1],
                scalar2=None,
                op0=OP.mult,
            )

        # ---------------- w^T via PE transpose ---------------------------
        wT = [const.tile([P, N], F32, tag=f"wT{c}", name=f"wT{c}") for c in range(2)]
        for ci in range(2):
            for cj in range(2):
                tp = ppre.tile([P, P], F32, tag="tp")
                nc.tensor.transpose(
                    tp, wgt[:, ci * N + cj * P : ci * N + (cj + 1) * P], ident
                )
                nc.scalar.copy(wT[cj][:, ci * P : (ci + 1) * P], tp)

        # ---------------- ctx^T[d,i] = sum_j x[j,d] w[i,j] ----------------
        ctxT = [const.tile([P, N], F32, tag=f"ctxT{c}", name=f"ctxT{c}") for c in range(2)]
        for dc in range(2):
            pc = ppre.tile([P, N], F32, tag="mm")
            for jc in range(2):
                nc.tensor.matmul(
                    pc,
                    x[jc][:, dc * P : (dc + 1) * P],
                    wT[jc],
                    start=(jc == 0),
                    stop=(jc == 1),
                )
            nc.scalar.copy(ctxT[dc], pc)

        # ---------------- proj / LayerNorm / residual ---------------------
        for icc in range(2):
            pp = ppre.tile([P, N], F32, tag="mm")
            for dc in range(2):
                nc.tensor.matmul(
                    pp,
                    ctxT[dc][:, icc * P : (icc + 1) * P],
                    wp[dc],
                    start=(dc == 0),
                    stop=(dc == 1),
                )
            pb = const.tile([P, D], F32, tag=f"pb{icc}", name=f"pb{icc}")
            nc.vector.tensor_tensor(out=pb, in0=pp, in1=bpr, op=OP.add)
            st = const.tile([P, 6], F32, tag=f"st{icc}", name=f"st{icc}")
            nc.vector.bn_stats(st, pb)
            mv = const.tile([P, 2], F32, tag=f"mv{icc}", name=f"mv{icc}")
            nc.vector.bn_aggr(mv, st)
            lnv = const.tile([P, 1], F32, tag=f"lnv{icc}", name=f"lnv{icc}")
            nc.scalar.activation(lnv, mv[:, 1:2], AF.Ln, bias=eps1[:, 0:1])
            rstd = const.tile([P, 1], F32, tag=f"rstd{icc}", name=f"rstd{icc}")
            nc.scalar.activation(rstd, lnv, AF.Exp, bias=zero1[:, 0:1], scale=-0.5)
            nmr = const.tile([P, 1], F32, tag=f"nmr{icc}", name=f"nmr{icc}")
            nc.vector.tensor_scalar(
                out=nmr,
                in0=mv[:, 0:1],
                scalar1=rstd[:, 0:1],
                scalar2=-1.0,
                op0=OP.mult,
                op1=OP.mult,
            )
            tt = const.tile([P, D], F32, tag=f"tt{icc}", name=f"tt{icc}")
            nc.vector.tensor_scalar(
                out=tt,
                in0=pb,
                scalar1=rstd[:, 0:1],
                scalar2=nmr[:, 0:1],
                op0=OP.mult,
                op1=OP.add,
            )
            if use_gamma:
                tg = const.tile([P, D], F32, tag=f"tg{icc}", name=f"tg{icc}")
                nc.gpsimd.tensor_tensor(out=tg, in0=tt, in1=gam, op=OP.mult)
                tt = tg
            ot = const.tile([P, D], F32, tag=f"ot{icc}", name=f"ot{icc}")
            nc.gpsimd.tensor_tensor(out=ot, in0=tt, in1=xpb[icc], op=OP.add)
            nc.sync.dma_start(out_d[icc * P : (icc + 1) * P, :], ot)

    nc.compile()
    return nc


_cache = {}


def _get_program(b2_val: float, use_gamma: bool, use_beta: bool):
    key = (b2_val, use_gamma, use_beta)
    if key not in _cache:
        _cache[key] = _build_program(b2_val, use_gamma, use_beta)
    return _cache[key]


def _host_inputs(inputs):
    x = np.ascontiguousarray(np.asarray(inputs["patch_features"], np.float32))
    W1 = np.asarray(inputs["W1"], np.float32)
    b1 = np.asarray(inputs["b1"], np.float32)
    W2 = np.asarray(inputs["W2"], np.float32).reshape(-1)
    b2 = float(np.asarray(inputs["b2"], np.float32).reshape(-1)[0])
    Wp = np.ascontiguousarray(np.asarray(inputs["Wp"], np.float32))
    bp = np.asarray(inputs["bp"], np.float32)
    gam = np.asarray(inputs["ln_gamma"], np.float32)
    bet = np.asarray(inputs["ln_beta"], np.float32)

    a = np.abs(W2)
    sig = np.where(W2 >= 0.0, 1.0, -1.0).astype(np.float32)
    w1a = np.ascontiguousarray((W1[:D] * a[None, :]).astype(np.float16))
    w1b = np.ascontiguousarray((W1[D:] * a[None, :]).astype(np.float16))
    ab1c = np.ascontiguousarray((a * b1).reshape(2, P).T)  # [P, 2]
    sbs = []
    for c in range(2):
        m = np.zeros((P, 64), np.float16)
        m[:, 32] = sig[c * P : (c + 1) * P].astype(np.float16)
        sbs.append(m)
    mask = np.ones((P, 2 * N), np.float32)
    for p in range(P):
        mask[p, p] = 0.0
        mask[p, N + P + p] = 0.0
    ident = np.eye(P, dtype=np.float32)
    bpr = np.ascontiguousarray(np.broadcast_to(bp[None, :], (P, D)))
    use_gamma = not np.all(gam == 1.0)
    use_beta = not np.all(bet == 0.0)
    gamr = np.ascontiguousarray(np.broadcast_to(gam[None, :], (P, D)))

    common = {
        "w1a": w1a,
        "w1b": w1b,
        "ab1c": ab1c,
        "sb0": sbs[0],
        "sb1": sbs[1],
        "maskd": mask,
        "ident": ident,
        "wp": Wp,
        "bpr": bpr,
    }
    if use_gamma:
        common["gamr"] = gamr
    in_maps = []
    for b in range(B):
        m = dict(common)
        m["xb"] = np.ascontiguousarray(x[b])
        if use_beta:
            m["xpb"] = np.ascontiguousarray(x[b] + bet[None, :])
        in_maps.append(m)
    return in_maps, b2, use_gamma, use_beta


def _run(inputs, trace=False, tmpdir=None):
    in_maps, b2, use_gamma, use_beta = _host_inputs(inputs)
    nc = _get_program(b2, use_gamma, use_beta)
    res = run_bass_kernel_spmd(
        nc, in_maps, list(range(B)), trace=trace, tmpdir=tmpdir
    )
    out = np.stack([res.results[b]["out"] for b in range(B)]).astype(np.float32)
    return out, res


def kernel(**inputs) -> np.ndarray:
    out, _ = _run(inputs)
    return out


def predicted_time_ns():
    """Cost-model timeline estimate of one core's NEFF execution (ns)."""
    from concourse.timeline_sim import TimelineSim

    assert _cache, "run the kernel first"
    nc = next(iter(_cache.values()))
    tl = TimelineSim(nc, trace=False)
    return int(tl.simulate())
